# revision 14
# baseline (speedup 1.0000x reference)
"""Block-diagonal GRU cell on 8 TRN2 NeuronCores — one block per core.

Math per block n (torch GRUCell):
  gi = x_n @ W_ih[n].T + b_ih[n]        (B, 3*BS)
  gh = h_n @ W_hh[n].T + b_hh[n]
  r = sigmoid(gi_r + gh_r); z = sigmoid(gi_z + gh_z)
  ng = tanh(gi_n + r * gh_n)
  h' = ng + z * (h_n - ng)

On-chip layout (per core): everything transposed on host so the
contraction (feature) dim is the SBUF partition dim and gates land on
PSUM partitions — biases then apply as per-partition ACT/DVE operands.
  A  = [W_ih[n].T ; W_hh[n].T]  -> (1024 feat, 1536 gates) bf16, cut
       into 96 (128x128) k-tiles laid out in DRAM in the EXACT order the
       PE consumes them, so every load is a contiguous prefix of the
       consumption stream and arbitrary chunk boundaries stay aligned
       with compute progress.
  U  = [x_n.T ; h_n.T]          -> (1024 feat, 1024 batch) bf16
  out = h'.T                    -> (512, 1024) bf16, un-transposed and
       upcast on host.
All matmuls run in bf16 (fp8 was measured 2.4-5x over the error budget
in simulation). r/z gates accumulate x- and h-matmuls into one PSUM
bank (8 k-steps); the n gate keeps i_n / h_n in separate banks.
Loads ride BOTH HWDGE rings: the Scalar (qActDynamicHW) ring carries
the first critical chunks (its engine clears the preamble barrier
~0.9us before Sync does) and the Sync (qSPDynamicHW) ring streams the
bulk — the two rings issue triggers in parallel so small early chunks
don't serialize behind one engine's ~0.65us/trigger cost. Early chunks
are small (128-384KB) to make the first matmul's data land ASAP; later
chunks are big for bandwidth. Warm-up matmuls (N=128 off a memset
tile) keep the PE HAM activity window busy from the top of the kernel
until real data lands, so the 1.2->2.4GHz un-throttle fires during the
fill instead of 8us into the real stream. The final row-block group
runs its z-gate matmuls last in 128-wide quarters so the chain
trailing the very last matmul is sigmoid->mul->add->small-store, with
stores alternating rings.
"""

import os
import sys

import numpy as np

try:
    import concourse.bass as bass
except ImportError:  # fresh grading dir: fall back to the repo checkout
    sys.path.insert(0, "/opt/trn_rl_repo")
    import concourse.bass as bass

import concourse.mybir as mybir
import concourse.tile as tile
from concourse import bacc
from concourse.bass import ts
from concourse.bass_utils import run_bass_kernel_spmd

B = 1024            # batch
NB = 8              # blocks == cores
BS = 512            # hidden block size
G3 = 3 * BS         # gates per block (r, z, n)
KF = 1024           # contraction feats per core: 512 input + 512 hidden
P = 128
KT = KF // P        # 8 k-tiles
GT = G3 // P        # 12 gate column groups: 0-3 r, 4-7 z, 8-11 n
NBC = 2             # batch chunks
BC = B // NBC       # 512 (one PSUM bank of fp32)
NROW = GT * KT      # 96 (128x128) A tiles

F32 = mybir.dt.float32
BF16 = mybir.dt.bfloat16
AFT = mybir.ActivationFunctionType
ALU = mybir.AluOpType

_cache: dict = {}
LAST_RESULTS = None  # BassKernelResults of the most recent run (for test.py)


def _row_order():
    """(gate_group, k) per DRAM row of A, in exact PE consumption order.

    j=0 runs k-major (r,z,n per k-tile) so each 128KB U k-tile plus
    three 32KB A rows unlocks three matmuls — the finest-grained
    data->compute pipeline the DMA sem granularity allows; j=1..3 run
    r(k0-7), z(k0-7), n(k4-7), n(k0-3) — n's k4-7 half first because
    the tanh chain consumes the h-side accumulation before the i-side.
    """
    rows = []
    for k in range(8):
        rows += [(0, k), (4, k), (8, k)]
    for j in (1, 2, 3):
        rows += [(j, k) for k in range(8)]
        rows += [(4 + j, k) for k in range(8)]
        rows += [(8 + j, k) for k in range(4, 8)]
        rows += [(8 + j, k) for k in range(4)]
    assert len(rows) == NROW
    return rows


ROW_ORDER = _row_order()
ROW_IDX = {gk: i for i, gk in enumerate(ROW_ORDER)}

# Fill rides TWO descriptor rings whose streams are CO-critical and
# aligned with the interleaved consumption: the U k-tiles on the Sync
# HWDGE ring, the j0 A rows on the GpSimd SWDGE ring (its engine clears
# the preamble barrier ~0.6us before Sync). Per-chunk trigger (~0.65us)
# and completion-receipt (~0.7us) overheads then pipeline in parallel,
# which single-ring delivery (~250-300GB/s effective vs the 346GB/s the
# warm j0 stream consumes) could not sustain. Bulk-starving-the-front
# (the earlier 2-ring failure) doesn't apply because both rings carry
# front-of-stream data in matching order. j1..j3 A chunks go on Sync
# after the U k-tiles; chunk sizes grow once the stream is ahead.
GPSIMD_LOADS = [
    ("a", 0, 3), ("a", 3, 6), ("a", 6, 12), ("a", 12, 18), ("a", 18, 24),
    ("a", 24, 32), ("a", 32, 40), ("a", 40, 48),
]
SYNC_LOADS = [
    ("u", 0, 0, 1), ("u", 0, 1, 2), ("u", 0, 2, 4),
    ("u", 0, 4, 6), ("u", 0, 6, 8),
    ("a", 48, 72), ("a", 72, 96),
    ("u", 1, 0, 8),
]

# Bridge PE activity from barrier-exit (~7.4us abs) past first-data
# (~9.5us abs) with margin — an idle hole here resets the HAM activity
# window and costs ~4us of 1.2GHz matmuls (measured).
N_WARMUP = 26


def _build_nc():
    nc = bacc.Bacc("TRN2", target_bir_lowering=False, debug=False, num_devices=NB)
    a_d = nc.dram_tensor("a", [P, NROW, P], BF16, kind="ExternalInput").ap()
    u_d = nc.dram_tensor("u", [P, NBC, KT, BC], BF16, kind="ExternalInput").ap()
    brz_d = nc.dram_tensor("brz", [P, 12], F32, kind="ExternalInput").ap()
    bn_d = nc.dram_tensor("bn", [P, 8], F32, kind="ExternalInput").ap()
    o_d = nc.dram_tensor("o", [BS, B], BF16, kind="ExternalOutput").ap()

    with tile.TileContext(nc) as tc:
        with (
            tc.tile_pool(name="persist", bufs=1) as persist,
            tc.tile_pool(name="tmp", bufs=4) as tmp,
            tc.tile_pool(name="outp", bufs=4) as outp,
            tc.tile_pool(name="psum", bufs=8, space="PSUM") as psum,
        ):
            # Warm-up source: memset on Vector (fast, idle; a GpSimd memset
            # of [128,128] measured ~1.2us and serialized both the warm-ups
            # and the GpSimd A-ring triggers behind it).
            wsb = persist.tile([P, P], BF16, name="wsb")
            nc.vector.memset(wsb[:], 0.0)

            U = persist.tile([P, NBC, KT, BC], BF16, name="U")
            A = persist.tile([P, NROW, P], BF16, name="A")

            def emit_loads(eng, chunks):
                for c in chunks:
                    if c[0] == "a":
                        _, r0, r1 = c
                        eng.dma_start(A[:, r0:r1, :], a_d[:, r0:r1, :])
                    else:
                        _, bcq, k0, k1 = c
                        eng.dma_start(U[:, bcq, k0:k1, :], u_d[:, bcq, k0:k1, :])

            emit_loads(nc.gpsimd, GPSIMD_LOADS)
            emit_loads(nc.sync, SYNC_LOADS)

            # tiny bias loads ride the GpSimd SWDGE queue after the
            # critical j0 A rows; first needed at j0's sigmoid (~13us)
            brz_sb = persist.tile([P, 12], F32, name="brz_sb")
            nc.gpsimd.dma_start(brz_sb[:], brz_d[:])
            bn_sb = persist.tile([P, 8], F32, name="bn_sb")
            nc.gpsimd.dma_start(bn_sb[:], bn_d[:])

            # PE warm-up matmuls: cover the HAM activity window + data-fill
            # lead so the first real matmuls run at 2.4GHz.
            wps = psum.tile([P, BC], F32, name="wps", tag="ps")
            for _ in range(N_WARMUP):
                nc.tensor.matmul(wps[:, :P], wsb[:], wsb[:], start=True, stop=True)

            def lhsT(g, k):
                return A[:, ROW_IDX[(g, k)], :]

            # persistent per row-block j: r gate, z gate (bf16)
            r_t = [persist.tile([P, B], BF16, name=f"r{j}") for j in range(4)]
            z_t = [persist.tile([P, B], BF16, name=f"z{j}") for j in range(4)]

            def mm_group(g, bc, c0, w, k0, k1):
                ps = psum.tile([P, w], F32, name="ps", tag="ps")
                for k in range(k0, k1):
                    nc.tensor.matmul(
                        ps[:],
                        lhsT(g, k),
                        U[:, bc, k, c0 : c0 + w],
                        start=(k == k0),
                        stop=(k == k1 - 1),
                    )
                return ps

            def make_nt(j, bc, c0, w, ps_i, ps_h, sl):
                # ng = tanh(i_n + b_in + r*(h_n + b_hn)); d = h - ng
                t = tmp.tile([P, w], F32, name="t", tag="t")
                nc.vector.scalar_tensor_tensor(
                    t[:], ps_h[:, sl], bn_sb[:, 4 + j : 5 + j],
                    r_t[j][:, c0 : c0 + w], ALU.add, ALU.mult,
                )
                t2 = tmp.tile([P, w], BF16, name="t2", tag="t2")
                nc.vector.tensor_add(t2[:], t[:], ps_i[:, sl])
                nt = tmp.tile([P, w], BF16, name="nt", tag="nt")
                nc.scalar.activation(nt[:], t2[:], AFT.Tanh, bias=bn_sb[:, j : j + 1])
                d = tmp.tile([P, w], BF16, name="d", tag="d")
                nc.vector.tensor_sub(
                    d[:], U[:, bc, 4 + j, c0 - bc * BC : c0 - bc * BC + w], nt[:]
                )
                return nt, d

            def combine(j, c0, w, nt, d, ntsl, ring=None, veng=None):
                # h' = ng + z*(h - ng): only two serial DVE ops after z
                veng = veng or nc.vector
                zd = tmp.tile([P, w], BF16, name="zd", tag="zd")
                veng.tensor_mul(zd[:], z_t[j][:, c0 : c0 + w], d[:, ntsl])
                o_t = outp.tile([P, w], BF16, name="o_t", tag="o_t")
                veng.tensor_add(o_t[:], nt[:, ntsl], zd[:])
                (ring or nc.sync).dma_start(o_d[ts(j, P), c0 : c0 + w], o_t[:])

            def interleaved_group(j, bc):
                # first group only: k-major so each (U k-tile, 3 A rows)
                # chunk unlocks 3 matmuls in arrival order (PE queue is
                # in-order)
                ps_r = psum.tile([P, BC], F32, name="ps", tag="ps")
                ps_z = psum.tile([P, BC], F32, name="ps", tag="ps")
                ps_i = psum.tile([P, BC], F32, name="ps", tag="ps")
                ps_h = psum.tile([P, BC], F32, name="ps", tag="ps")

                def mm(ps, g, k, start, stop):
                    nc.tensor.matmul(ps[:], lhsT(g, k), U[:, bc, k, :],
                                     start=start, stop=stop)

                for k in range(8):
                    mm(ps_r, j, k, k == 0, k == 7)
                    mm(ps_z, 4 + j, k, k == 0, k == 7)
                    if k < 4:
                        mm(ps_i, 8 + j, k, k == 0, k == 3)
                    else:
                        mm(ps_h, 8 + j, k, k == 4, k == 7)
                return ps_r, ps_z, ps_i, ps_h

            for bc in range(NBC):
                for j in range(4):
                    last = bc == NBC - 1 and j == 3
                    if bc == 0 and j == 0:
                        ps_r, ps_z, ps_i, ps_h = interleaved_group(j, bc)
                        nc.scalar.activation(
                            r_t[j][:, ts(bc, BC)], ps_r[:], AFT.Sigmoid,
                            bias=brz_sb[:, j : j + 1],
                        )
                        nc.scalar.activation(
                            z_t[j][:, ts(bc, BC)], ps_z[:], AFT.Sigmoid,
                            bias=brz_sb[:, 4 + j : 5 + j],
                        )
                        nt, dd = make_nt(j, bc, bc * BC, BC, ps_i, ps_h, slice(0, BC))
                        combine(j, bc * BC, BC, nt, dd, slice(0, BC))
                        # pad the PSUM ring to 8 allocations (wps + j0's 4
                        # + 3) so every later group's 4 tiles recycle the
                        # SAME-role tiles of the group two back (ps_r <-
                        # ps_r etc.); misaligned, a group's first matmul
                        # waits on the z-sigmoid of two groups ago
                        # (measured 432ns every other group)
                        for _ in range(3):
                            psum.tile([P, BC], F32, name="pad", tag="ps")
                        continue
                    ps_r = mm_group(j, bc, 0, BC, 0, KT)
                    nc.scalar.activation(
                        r_t[j][:, ts(bc, BC)], ps_r[:], AFT.Sigmoid,
                        bias=brz_sb[:, j : j + 1],
                    )
                    if not last:
                        ps_z = mm_group(4 + j, bc, 0, BC, 0, KT)
                        nc.scalar.activation(
                            z_t[j][:, ts(bc, BC)], ps_z[:], AFT.Sigmoid,
                            bias=brz_sb[:, 4 + j : 5 + j],
                        )
                        ps_h = mm_group(8 + j, bc, 0, BC, 4, KT)
                        ps_i = mm_group(8 + j, bc, 0, BC, 0, 4)
                        nt, dd = make_nt(j, bc, bc * BC, BC, ps_i, ps_h, slice(0, BC))
                        combine(j, bc * BC, BC, nt, dd, slice(0, BC))
                    else:
                        # final group: z gate LAST, in 128-wide quarters, so
                        # the chain trailing the very last matmul is a short
                        # sigmoid->mul->add->small-store. tanh/sub run in
                        # 256-halves so d's first half is ready before the
                        # first combine needs it; combines alternate
                        # Vector/GpSimd so the 8 elementwise ops don't
                        # serialize on one engine; stores alternate rings.
                        ps_h = mm_group(8 + j, bc, 0, BC, 4, KT)
                        ps_i = mm_group(8 + j, bc, 0, BC, 0, 4)
                        nts, dds = [], []
                        for hh in range(2):
                            hsl = slice(hh * 256, hh * 256 + 256)
                            nt_h, dd_h = make_nt(
                                j, bc, bc * BC + hh * 256, 256,
                                ps_i, ps_h, hsl,
                            )
                            nts.append(nt_h)
                            dds.append(dd_h)
                        # all quarter matmuls + sigmoids first (Scalar runs
                        # them back-to-back as MM groups complete), THEN the
                        # combines — a store on Scalar's FIFO ahead of a
                        # later sigmoid would block it
                        for q in range(4):
                            off = q * P
                            c0 = bc * BC + off
                            ps_z = mm_group(4 + j, bc, off, P, 0, KT)
                            nc.scalar.activation(
                                z_t[j][:, c0 : c0 + P], ps_z[:],
                                AFT.Sigmoid, bias=brz_sb[:, 4 + j : 5 + j],
                            )
                        for q in range(4):
                            off = q * P
                            c0 = bc * BC + off
                            sl = slice((q % 2) * P, (q % 2) * P + P)
                            ring = nc.scalar if q % 2 == 0 else nc.sync
                            combine(j, c0, P, nts[q // 2], dds[q // 2], sl,
                                    ring=ring)

    nc.compile()
    return nc


def _prep_core_inputs(x16, h16, W_ih16, W_hh16, b_ih, b_hh, n):
    a_full = np.concatenate([W_ih16[n].T, W_hh16[n].T], axis=0)      # (1024, 1536)
    a4 = a_full.reshape(KT, P, GT, P)                                # [k, kp, g, gp]
    a_rows = np.stack([a4[k, :, g, :] for (g, k) in ROW_ORDER])      # (96, P, P)
    a_re = np.ascontiguousarray(a_rows.transpose(1, 0, 2))           # (P, 96, P)
    u = np.ascontiguousarray(
        np.concatenate(
            [x16[:, n * BS : (n + 1) * BS].T, h16[:, n * BS : (n + 1) * BS].T],
            axis=0,
        ).reshape(KT, P, NBC, BC).transpose(1, 2, 0, 3)
    )                                                                # (P, NBC, KT, BC)
    brz8 = (b_ih[n, : 2 * BS] + b_hh[n, : 2 * BS]).reshape(8, P).T   # (P, 8)
    brz = np.ascontiguousarray(
        np.concatenate([brz8, -brz8[:, 4:8]], axis=1)
    )                                                                # (P, 12)
    bn = np.ascontiguousarray(
        np.concatenate(
            [b_ih[n, 2 * BS :].reshape(4, P).T, b_hh[n, 2 * BS :].reshape(4, P).T],
            axis=1,
        )
    )                                                                # (P, 8)
    return {"a": a_re, "u": u, "brz": brz, "bn": bn}


def kernel(x, h, W_ih, W_hh, b_ih, b_hh):
    global LAST_RESULTS
    import ml_dtypes

    bf16 = np.dtype(ml_dtypes.bfloat16)
    x16 = np.asarray(x, dtype=np.float32).astype(bf16)
    h16 = np.asarray(h, dtype=np.float32).astype(bf16)
    W_ih16 = np.asarray(W_ih, dtype=np.float32).astype(bf16)
    W_hh16 = np.asarray(W_hh, dtype=np.float32).astype(bf16)
    b_ih = np.asarray(b_ih, dtype=np.float32)
    b_hh = np.asarray(b_hh, dtype=np.float32)

    if "nc" not in _cache:
        _cache["nc"] = _build_nc()
    nc = _cache["nc"]

    in_maps = [
        _prep_core_inputs(x16, h16, W_ih16, W_hh16, b_ih, b_hh, n)
        for n in range(NB)
    ]
    trace = os.environ.get("BASS_KERNEL_TRACE") == "1"
    res = run_bass_kernel_spmd(nc, in_maps, list(range(NB)), trace=trace)
    LAST_RESULTS = res
    return np.concatenate(
        [res.results[n]["o"].astype(np.float32).T for n in range(NB)], axis=1
    )


# revision 16
# speedup vs baseline: 1.0336x; 1.0336x over previous
"""Block-diagonal GRU cell on 8 TRN2 NeuronCores — one block per core.

Math per block n (torch GRUCell):
  gi = x_n @ W_ih[n].T + b_ih[n]        (B, 3*BS)
  gh = h_n @ W_hh[n].T + b_hh[n]
  r = sigmoid(gi_r + gh_r); z = sigmoid(gi_z + gh_z)
  ng = tanh(gi_n + r * gh_n)
  h' = ng + z * (h_n - ng)

On-chip layout (per core): everything transposed on host so the
contraction (feature) dim is the SBUF partition dim and gates land on
PSUM partitions — biases then apply as per-partition ACT/DVE operands.
  A  = [W_ih[n].T ; W_hh[n].T]  -> (1024 feat, 1536 gates) bf16, cut
       into 96 (128x128) k-tiles laid out in DRAM in the EXACT order the
       PE consumes them, so every load is a contiguous prefix of the
       consumption stream and arbitrary chunk boundaries stay aligned
       with compute progress.
  U  = [x_n.T ; h_n.T]          -> (1024 feat, 1024 batch) bf16
  out = h'.T                    -> (512, 1024) bf16, un-transposed and
       upcast on host.
All matmuls run in bf16 (fp8 was measured 2.4-5x over the error budget
in simulation). r/z gates accumulate x- and h-matmuls into one PSUM
bank (8 k-steps); the n gate keeps i_n / h_n in separate banks.
Loads ride BOTH HWDGE rings: the Scalar (qActDynamicHW) ring carries
the first critical chunks (its engine clears the preamble barrier
~0.9us before Sync does) and the Sync (qSPDynamicHW) ring streams the
bulk — the two rings issue triggers in parallel so small early chunks
don't serialize behind one engine's ~0.65us/trigger cost. Early chunks
are small (128-384KB) to make the first matmul's data land ASAP; later
chunks are big for bandwidth. Warm-up matmuls (N=128 off a memset
tile) keep the PE HAM activity window busy from the top of the kernel
until real data lands, so the 1.2->2.4GHz un-throttle fires during the
fill instead of 8us into the real stream. The final row-block group
runs its z-gate matmuls last in 128-wide quarters so the chain
trailing the very last matmul is sigmoid->mul->add->small-store, with
stores alternating rings.
"""

import os
import sys

import numpy as np

try:
    import concourse.bass as bass
except ImportError:  # fresh grading dir: fall back to the repo checkout
    sys.path.insert(0, "/opt/trn_rl_repo")
    import concourse.bass as bass

import concourse.mybir as mybir
import concourse.tile as tile
from concourse import bacc
from concourse.bass import ts
from concourse.bass_utils import run_bass_kernel_spmd

B = 1024            # batch
NB = 8              # blocks == cores
BS = 512            # hidden block size
G3 = 3 * BS         # gates per block (r, z, n)
KF = 1024           # contraction feats per core: 512 input + 512 hidden
P = 128
KT = KF // P        # 8 k-tiles
GT = G3 // P        # 12 gate column groups: 0-3 r, 4-7 z, 8-11 n
NBC = 2             # batch chunks
BC = B // NBC       # 512 (one PSUM bank of fp32)
NROW = GT * KT      # 96 (128x128) A tiles

F32 = mybir.dt.float32
BF16 = mybir.dt.bfloat16
AFT = mybir.ActivationFunctionType
ALU = mybir.AluOpType

_cache: dict = {}
LAST_RESULTS = None  # BassKernelResults of the most recent run (for test.py)


def _row_order():
    """(gate_group, k) per DRAM row of A, in exact PE consumption order.

    j=0 runs k-major (r,z,n per k-tile) so each 128KB U k-tile plus
    three 32KB A rows unlocks three matmuls — the finest-grained
    data->compute pipeline the DMA sem granularity allows; j=1..3 run
    r(k0-7), z(k0-7), n(k4-7), n(k0-3) — n's k4-7 half first because
    the tanh chain consumes the h-side accumulation before the i-side.
    """
    rows = []
    for k in range(8):
        rows += [(0, k), (4, k), (8, k)]
    for j in (1, 2, 3):
        rows += [(j, k) for k in range(8)]
        rows += [(4 + j, k) for k in range(8)]
        rows += [(8 + j, k) for k in range(4, 8)]
        rows += [(8 + j, k) for k in range(4)]
    assert len(rows) == NROW
    return rows


ROW_ORDER = _row_order()
ROW_IDX = {gk: i for i, gk in enumerate(ROW_ORDER)}

# Fill rides TWO descriptor rings whose streams are CO-critical and
# aligned with the interleaved consumption: the U k-tiles on the Sync
# HWDGE ring, the j0 A rows on the GpSimd SWDGE ring (its engine clears
# the preamble barrier ~0.6us before Sync). Per-chunk trigger (~0.65us)
# and completion-receipt (~0.7us) overheads then pipeline in parallel,
# which single-ring delivery (~250-300GB/s effective vs the 346GB/s the
# warm j0 stream consumes) could not sustain. Bulk-starving-the-front
# (the earlier 2-ring failure) doesn't apply because both rings carry
# front-of-stream data in matching order. j1..j3 A chunks go on Sync
# after the U k-tiles; chunk sizes grow once the stream is ahead.
GPSIMD_LOADS = [
    ("a", 0, 3), ("a", 3, 6), ("a", 6, 12), ("a", 12, 18), ("a", 18, 24),
]
SYNC_LOADS = [
    ("u", 0, 0, 2), ("u", 0, 2, 4), ("u", 0, 4, 6), ("u", 0, 6, 8),
    ("a", 24, 36), ("a", 36, 48), ("a", 48, 72), ("a", 72, 96),
    ("u", 1, 0, 8),
]

# Bridge PE activity from barrier-exit (~7.4us abs) past first-data
# (~9.5us abs) with margin — an idle hole here resets the HAM activity
# window and costs ~4us of 1.2GHz matmuls (measured).
N_WARMUP = 26


def _build_nc():
    nc = bacc.Bacc("TRN2", target_bir_lowering=False, debug=False, num_devices=NB)
    a_d = nc.dram_tensor("a", [P, NROW, P], BF16, kind="ExternalInput").ap()
    u_d = nc.dram_tensor("u", [P, NBC, KT, BC], BF16, kind="ExternalInput").ap()
    brz_d = nc.dram_tensor("brz", [P, 12], F32, kind="ExternalInput").ap()
    bn_d = nc.dram_tensor("bn", [P, 8], F32, kind="ExternalInput").ap()
    o_d = nc.dram_tensor("o", [BS, B], BF16, kind="ExternalOutput").ap()

    with tile.TileContext(nc) as tc:
        with (
            tc.tile_pool(name="persist", bufs=1) as persist,
            tc.tile_pool(name="tmp", bufs=4) as tmp,
            tc.tile_pool(name="outp", bufs=4) as outp,
            tc.tile_pool(name="psum", bufs=8, space="PSUM") as psum,
        ):
            # Warm-up source: memset on Vector (fast, idle; a GpSimd memset
            # of [128,128] measured ~1.2us and serialized both the warm-ups
            # and the GpSimd A-ring triggers behind it).
            wsb = persist.tile([P, P], BF16, name="wsb")
            nc.vector.memset(wsb[:], 0.0)

            U = persist.tile([P, NBC, KT, BC], BF16, name="U")
            A = persist.tile([P, NROW, P], BF16, name="A")

            def emit_loads(eng, chunks):
                for c in chunks:
                    if c[0] == "a":
                        _, r0, r1 = c
                        eng.dma_start(A[:, r0:r1, :], a_d[:, r0:r1, :])
                    else:
                        _, bcq, k0, k1 = c
                        eng.dma_start(U[:, bcq, k0:k1, :], u_d[:, bcq, k0:k1, :])

            emit_loads(nc.gpsimd, GPSIMD_LOADS)
            emit_loads(nc.sync, SYNC_LOADS)

            # tiny bias loads ride the GpSimd SWDGE queue after the
            # critical j0 A rows; first needed at j0's sigmoid (~13us)
            brz_sb = persist.tile([P, 12], F32, name="brz_sb")
            nc.gpsimd.dma_start(brz_sb[:], brz_d[:])
            bn_sb = persist.tile([P, 8], F32, name="bn_sb")
            nc.gpsimd.dma_start(bn_sb[:], bn_d[:])

            # PE warm-up matmuls: cover the HAM activity window + data-fill
            # lead so the first real matmuls run at 2.4GHz.
            wps = psum.tile([P, BC], F32, name="wps", tag="ps")
            for _ in range(N_WARMUP):
                nc.tensor.matmul(wps[:, :P], wsb[:], wsb[:], start=True, stop=True)

            def lhsT(g, k):
                return A[:, ROW_IDX[(g, k)], :]

            # persistent per row-block j: r gate, z gate (bf16)
            r_t = [persist.tile([P, B], BF16, name=f"r{j}") for j in range(4)]
            z_t = [persist.tile([P, B], BF16, name=f"z{j}") for j in range(4)]

            def mm_group(g, bc, c0, w, k0, k1):
                ps = psum.tile([P, w], F32, name="ps", tag="ps")
                for k in range(k0, k1):
                    nc.tensor.matmul(
                        ps[:],
                        lhsT(g, k),
                        U[:, bc, k, c0 : c0 + w],
                        start=(k == k0),
                        stop=(k == k1 - 1),
                    )
                return ps

            def make_nt(j, bc, c0, w, ps_i, ps_h, sl):
                # ng = tanh(i_n + b_in + r*(h_n + b_hn)); d = h - ng
                t = tmp.tile([P, w], F32, name="t", tag="t")
                nc.vector.scalar_tensor_tensor(
                    t[:], ps_h[:, sl], bn_sb[:, 4 + j : 5 + j],
                    r_t[j][:, c0 : c0 + w], ALU.add, ALU.mult,
                )
                t2 = tmp.tile([P, w], BF16, name="t2", tag="t2")
                nc.vector.tensor_add(t2[:], t[:], ps_i[:, sl])
                nt = tmp.tile([P, w], BF16, name="nt", tag="nt")
                nc.scalar.activation(nt[:], t2[:], AFT.Tanh, bias=bn_sb[:, j : j + 1])
                d = tmp.tile([P, w], BF16, name="d", tag="d")
                nc.vector.tensor_sub(
                    d[:], U[:, bc, 4 + j, c0 - bc * BC : c0 - bc * BC + w], nt[:]
                )
                return nt, d

            def combine(j, c0, w, nt, d, ntsl, ring=None, veng=None):
                # h' = ng + z*(h - ng): only two serial DVE ops after z
                veng = veng or nc.vector
                zd = tmp.tile([P, w], BF16, name="zd", tag="zd")
                veng.tensor_mul(zd[:], z_t[j][:, c0 : c0 + w], d[:, ntsl])
                o_t = outp.tile([P, w], BF16, name="o_t", tag="o_t")
                veng.tensor_add(o_t[:], nt[:, ntsl], zd[:])
                (ring or nc.sync).dma_start(o_d[ts(j, P), c0 : c0 + w], o_t[:])

            def interleaved_group(j, bc):
                # first group only: k-major so each (U k-tile, 3 A rows)
                # chunk unlocks 3 matmuls in arrival order (PE queue is
                # in-order)
                ps_r = psum.tile([P, BC], F32, name="ps", tag="ps")
                ps_z = psum.tile([P, BC], F32, name="ps", tag="ps")
                ps_i = psum.tile([P, BC], F32, name="ps", tag="ps")
                ps_h = psum.tile([P, BC], F32, name="ps", tag="ps")

                def mm(ps, g, k, start, stop):
                    nc.tensor.matmul(ps[:], lhsT(g, k), U[:, bc, k, :],
                                     start=start, stop=stop)

                for k in range(8):
                    mm(ps_r, j, k, k == 0, k == 7)
                    mm(ps_z, 4 + j, k, k == 0, k == 7)
                    if k < 4:
                        mm(ps_i, 8 + j, k, k == 0, k == 3)
                    else:
                        mm(ps_h, 8 + j, k, k == 4, k == 7)
                return ps_r, ps_z, ps_i, ps_h

            for bc in range(NBC):
                for j in range(4):
                    last = bc == NBC - 1 and j == 3
                    if bc == 0 and j == 0:
                        ps_r, ps_z, ps_i, ps_h = interleaved_group(j, bc)
                        nc.scalar.activation(
                            r_t[j][:, ts(bc, BC)], ps_r[:], AFT.Sigmoid,
                            bias=brz_sb[:, j : j + 1],
                        )
                        nc.scalar.activation(
                            z_t[j][:, ts(bc, BC)], ps_z[:], AFT.Sigmoid,
                            bias=brz_sb[:, 4 + j : 5 + j],
                        )
                        nt, dd = make_nt(j, bc, bc * BC, BC, ps_i, ps_h, slice(0, BC))
                        combine(j, bc * BC, BC, nt, dd, slice(0, BC))
                        continue
                    ps_r = mm_group(j, bc, 0, BC, 0, KT)
                    nc.scalar.activation(
                        r_t[j][:, ts(bc, BC)], ps_r[:], AFT.Sigmoid,
                        bias=brz_sb[:, j : j + 1],
                    )
                    if not last:
                        ps_z = mm_group(4 + j, bc, 0, BC, 0, KT)
                        nc.scalar.activation(
                            z_t[j][:, ts(bc, BC)], ps_z[:], AFT.Sigmoid,
                            bias=brz_sb[:, 4 + j : 5 + j],
                        )
                        ps_h = mm_group(8 + j, bc, 0, BC, 4, KT)
                        ps_i = mm_group(8 + j, bc, 0, BC, 0, 4)
                        nt, dd = make_nt(j, bc, bc * BC, BC, ps_i, ps_h, slice(0, BC))
                        combine(j, bc * BC, BC, nt, dd, slice(0, BC))
                    else:
                        # final group: z gate LAST, in 128-wide quarters, so
                        # the chain trailing the very last matmul is a short
                        # sigmoid->mul->add->small-store. tanh/sub run in
                        # 256-halves so d's first half is ready before the
                        # first combine needs it; combines alternate
                        # Vector/GpSimd so the 8 elementwise ops don't
                        # serialize on one engine; stores alternate rings.
                        ps_h = mm_group(8 + j, bc, 0, BC, 4, KT)
                        ps_i = mm_group(8 + j, bc, 0, BC, 0, 4)
                        nts, dds = [], []
                        for hh in range(2):
                            hsl = slice(hh * 256, hh * 256 + 256)
                            nt_h, dd_h = make_nt(
                                j, bc, bc * BC + hh * 256, 256,
                                ps_i, ps_h, hsl,
                            )
                            nts.append(nt_h)
                            dds.append(dd_h)
                        # all quarter matmuls + sigmoids first (Scalar runs
                        # them back-to-back as MM groups complete), THEN the
                        # combines — a store on Scalar's FIFO ahead of a
                        # later sigmoid would block it
                        for q in range(4):
                            off = q * P
                            c0 = bc * BC + off
                            ps_z = mm_group(4 + j, bc, off, P, 0, KT)
                            nc.scalar.activation(
                                z_t[j][:, c0 : c0 + P], ps_z[:],
                                AFT.Sigmoid, bias=brz_sb[:, 4 + j : 5 + j],
                            )
                        for q in range(4):
                            off = q * P
                            c0 = bc * BC + off
                            sl = slice((q % 2) * P, (q % 2) * P + P)
                            ring = nc.scalar if q % 2 == 0 else nc.sync
                            combine(j, c0, P, nts[q // 2], dds[q // 2], sl,
                                    ring=ring)

    nc.compile()
    return nc


def _prep_core_inputs(x16, h16, W_ih16, W_hh16, b_ih, b_hh, n):
    a_full = np.concatenate([W_ih16[n].T, W_hh16[n].T], axis=0)      # (1024, 1536)
    a4 = a_full.reshape(KT, P, GT, P)                                # [k, kp, g, gp]
    a_rows = np.stack([a4[k, :, g, :] for (g, k) in ROW_ORDER])      # (96, P, P)
    a_re = np.ascontiguousarray(a_rows.transpose(1, 0, 2))           # (P, 96, P)
    u = np.ascontiguousarray(
        np.concatenate(
            [x16[:, n * BS : (n + 1) * BS].T, h16[:, n * BS : (n + 1) * BS].T],
            axis=0,
        ).reshape(KT, P, NBC, BC).transpose(1, 2, 0, 3)
    )                                                                # (P, NBC, KT, BC)
    brz8 = (b_ih[n, : 2 * BS] + b_hh[n, : 2 * BS]).reshape(8, P).T   # (P, 8)
    brz = np.ascontiguousarray(
        np.concatenate([brz8, -brz8[:, 4:8]], axis=1)
    )                                                                # (P, 12)
    bn = np.ascontiguousarray(
        np.concatenate(
            [b_ih[n, 2 * BS :].reshape(4, P).T, b_hh[n, 2 * BS :].reshape(4, P).T],
            axis=1,
        )
    )                                                                # (P, 8)
    return {"a": a_re, "u": u, "brz": brz, "bn": bn}


def kernel(x, h, W_ih, W_hh, b_ih, b_hh):
    global LAST_RESULTS
    import ml_dtypes

    bf16 = np.dtype(ml_dtypes.bfloat16)
    x16 = np.asarray(x, dtype=np.float32).astype(bf16)
    h16 = np.asarray(h, dtype=np.float32).astype(bf16)
    W_ih16 = np.asarray(W_ih, dtype=np.float32).astype(bf16)
    W_hh16 = np.asarray(W_hh, dtype=np.float32).astype(bf16)
    b_ih = np.asarray(b_ih, dtype=np.float32)
    b_hh = np.asarray(b_hh, dtype=np.float32)

    if "nc" not in _cache:
        _cache["nc"] = _build_nc()
    nc = _cache["nc"]

    in_maps = [
        _prep_core_inputs(x16, h16, W_ih16, W_hh16, b_ih, b_hh, n)
        for n in range(NB)
    ]
    trace = os.environ.get("BASS_KERNEL_TRACE") == "1"
    res = run_bass_kernel_spmd(nc, in_maps, list(range(NB)), trace=trace)
    LAST_RESULTS = res
    return np.concatenate(
        [res.results[n]["o"].astype(np.float32).T for n in range(NB)], axis=1
    )


# revision 18
# speedup vs baseline: 1.0468x; 1.0128x over previous
"""Block-diagonal GRU cell on 8 TRN2 NeuronCores — one block per core.

Math per block n (torch GRUCell):
  gi = x_n @ W_ih[n].T + b_ih[n]        (B, 3*BS)
  gh = h_n @ W_hh[n].T + b_hh[n]
  r = sigmoid(gi_r + gh_r); z = sigmoid(gi_z + gh_z)
  ng = tanh(gi_n + r * gh_n)
  h' = ng + z * (h_n - ng)

On-chip layout (per core): everything transposed on host so the
contraction (feature) dim is the SBUF partition dim and gates land on
PSUM partitions — biases then apply as per-partition ACT/DVE operands.
  A  = [W_ih[n].T ; W_hh[n].T]  -> (1024 feat, 1536 gates) bf16, cut
       into 96 (128x128) k-tiles laid out in DRAM in the EXACT order the
       PE consumes them, so every load is a contiguous prefix of the
       consumption stream and arbitrary chunk boundaries stay aligned
       with compute progress.
  U  = [x_n.T ; h_n.T]          -> (1024 feat, 1024 batch) bf16
  out = h'.T                    -> (512, 1024) bf16, un-transposed and
       upcast on host.
All matmuls run in bf16 (fp8 was measured 2.4-5x over the error budget
in simulation). r/z gates accumulate x- and h-matmuls into one PSUM
bank (8 k-steps); the n gate keeps i_n / h_n in separate banks.
Loads ride BOTH HWDGE rings: the Scalar (qActDynamicHW) ring carries
the first critical chunks (its engine clears the preamble barrier
~0.9us before Sync does) and the Sync (qSPDynamicHW) ring streams the
bulk — the two rings issue triggers in parallel so small early chunks
don't serialize behind one engine's ~0.65us/trigger cost. Early chunks
are small (128-384KB) to make the first matmul's data land ASAP; later
chunks are big for bandwidth. Warm-up matmuls (N=128 off a memset
tile) keep the PE HAM activity window busy from the top of the kernel
until real data lands, so the 1.2->2.4GHz un-throttle fires during the
fill instead of 8us into the real stream. The final row-block group
runs its z-gate matmuls last in 128-wide quarters so the chain
trailing the very last matmul is sigmoid->mul->add->small-store, with
stores alternating rings.
"""

import os
import sys

import numpy as np

try:
    import concourse.bass as bass
except ImportError:  # fresh grading dir: fall back to the repo checkout
    sys.path.insert(0, "/opt/trn_rl_repo")
    import concourse.bass as bass

import concourse.mybir as mybir
import concourse.tile as tile
from concourse import bacc
from concourse.bass import ts
from concourse.bass_utils import run_bass_kernel_spmd

B = 1024            # batch
NB = 8              # blocks == cores
BS = 512            # hidden block size
G3 = 3 * BS         # gates per block (r, z, n)
KF = 1024           # contraction feats per core: 512 input + 512 hidden
P = 128
KT = KF // P        # 8 k-tiles
GT = G3 // P        # 12 gate column groups: 0-3 r, 4-7 z, 8-11 n
NBC = 2             # batch chunks
BC = B // NBC       # 512 (one PSUM bank of fp32)
NROW = GT * KT      # 96 (128x128) A tiles

F32 = mybir.dt.float32
BF16 = mybir.dt.bfloat16
AFT = mybir.ActivationFunctionType
ALU = mybir.AluOpType

_cache: dict = {}
LAST_RESULTS = None  # BassKernelResults of the most recent run (for test.py)


def _row_order():
    """(gate_group, k) per DRAM row of A, in exact PE consumption order.

    j=0 runs k-major (r,z,n per k-tile) so each 128KB U k-tile plus
    three 32KB A rows unlocks three matmuls — the finest-grained
    data->compute pipeline the DMA sem granularity allows; j=1..3 run
    r(k0-7), z(k0-7), n(k4-7), n(k0-3) — n's k4-7 half first because
    the tanh chain consumes the h-side accumulation before the i-side.
    """
    rows = []
    for k in range(8):
        rows += [(0, k), (4, k), (8, k)]
    for j in (1, 2, 3):
        rows += [(j, k) for k in range(8)]
        rows += [(4 + j, k) for k in range(8)]
        rows += [(8 + j, k) for k in range(4, 8)]
        rows += [(8 + j, k) for k in range(4)]
    assert len(rows) == NROW
    return rows


ROW_ORDER = _row_order()
ROW_IDX = {gk: i for i, gk in enumerate(ROW_ORDER)}

# Fill rides TWO descriptor rings whose streams are CO-critical and
# aligned with the interleaved consumption: the U k-tiles on the Sync
# HWDGE ring, the j0 A rows on the GpSimd SWDGE ring (its engine clears
# the preamble barrier ~0.6us before Sync). Per-chunk trigger (~0.65us)
# and completion-receipt (~0.7us) overheads then pipeline in parallel,
# which single-ring delivery (~250-300GB/s effective vs the 346GB/s the
# warm j0 stream consumes) could not sustain. Bulk-starving-the-front
# (the earlier 2-ring failure) doesn't apply because both rings carry
# front-of-stream data in matching order. j1..j3 A chunks go on Sync
# after the U k-tiles; chunk sizes grow once the stream is ahead.
GPSIMD_LOADS = [
    ("a", 0, 3), ("a", 3, 6), ("a", 6, 12), ("a", 12, 18), ("a", 18, 24),
]
SYNC_LOADS = [
    ("u", 0, 0, 2), ("u", 0, 2, 4), ("u", 0, 4, 6), ("u", 0, 6, 8),
    ("u", 1, 0, 2), ("u", 1, 2, 4), ("u", 1, 4, 6), ("u", 1, 6, 8),
    ("a", 24, 36), ("a", 36, 48), ("a", 48, 72), ("a", 72, 96),
]

# Bridge PE activity from barrier-exit (~7.4us abs) past first-data
# (~9.5us abs) with margin — an idle hole here resets the HAM activity
# window and costs ~4us of 1.2GHz matmuls (measured).
N_WARMUP = 26


def _build_nc():
    nc = bacc.Bacc("TRN2", target_bir_lowering=False, debug=False, num_devices=NB)
    a_d = nc.dram_tensor("a", [P, NROW, P], BF16, kind="ExternalInput").ap()
    u_d = nc.dram_tensor("u", [P, NBC, KT, BC], BF16, kind="ExternalInput").ap()
    brz_d = nc.dram_tensor("brz", [P, 12], F32, kind="ExternalInput").ap()
    bn_d = nc.dram_tensor("bn", [P, 8], F32, kind="ExternalInput").ap()
    o_d = nc.dram_tensor("o", [BS, B], BF16, kind="ExternalOutput").ap()

    with tile.TileContext(nc) as tc:
        with (
            tc.tile_pool(name="persist", bufs=1) as persist,
            tc.tile_pool(name="tmp", bufs=4) as tmp,
            tc.tile_pool(name="outp", bufs=4) as outp,
            tc.tile_pool(name="psum", bufs=8, space="PSUM") as psum,
        ):
            # Warm-up source: memset on Vector (fast, idle; a GpSimd memset
            # of [128,128] measured ~1.2us and serialized both the warm-ups
            # and the GpSimd A-ring triggers behind it).
            wsb = persist.tile([P, P], BF16, name="wsb")
            nc.vector.memset(wsb[:], 0.0)

            U = persist.tile([P, NBC, KT, BC], BF16, name="U")
            A = persist.tile([P, NROW, P], BF16, name="A")

            def emit_loads(eng, chunks):
                for c in chunks:
                    if c[0] == "a":
                        _, r0, r1 = c
                        eng.dma_start(A[:, r0:r1, :], a_d[:, r0:r1, :])
                    else:
                        _, bcq, k0, k1 = c
                        eng.dma_start(U[:, bcq, k0:k1, :], u_d[:, bcq, k0:k1, :])

            emit_loads(nc.gpsimd, GPSIMD_LOADS)
            emit_loads(nc.sync, SYNC_LOADS)

            # tiny bias loads ride the GpSimd SWDGE queue after the
            # critical j0 A rows; first needed at j0's sigmoid (~13us)
            brz_sb = persist.tile([P, 12], F32, name="brz_sb")
            nc.gpsimd.dma_start(brz_sb[:], brz_d[:])
            bn_sb = persist.tile([P, 8], F32, name="bn_sb")
            nc.gpsimd.dma_start(bn_sb[:], bn_d[:])

            # PE warm-up matmuls: cover the HAM activity window + data-fill
            # lead so the first real matmuls run at 2.4GHz.
            wps = psum.tile([P, BC], F32, name="wps", tag="ps")
            for _ in range(N_WARMUP):
                nc.tensor.matmul(wps[:, :P], wsb[:], wsb[:], start=True, stop=True)

            def lhsT(g, k):
                return A[:, ROW_IDX[(g, k)], :]

            # persistent per row-block j: r gate, z gate (bf16)
            r_t = [persist.tile([P, B], BF16, name=f"r{j}") for j in range(4)]
            z_t = [persist.tile([P, B], BF16, name=f"z{j}") for j in range(4)]

            def mm_group(g, bc, c0, w, k0, k1):
                ps = psum.tile([P, w], F32, name="ps", tag="ps")
                for k in range(k0, k1):
                    nc.tensor.matmul(
                        ps[:],
                        lhsT(g, k),
                        U[:, bc, k, c0 : c0 + w],
                        start=(k == k0),
                        stop=(k == k1 - 1),
                    )
                return ps

            def make_nt(j, bc, c0, w, ps_i, ps_h, sl):
                # ng = tanh(i_n + b_in + r*(h_n + b_hn)); d = h - ng
                t = tmp.tile([P, w], F32, name="t", tag="t")
                nc.vector.scalar_tensor_tensor(
                    t[:], ps_h[:, sl], bn_sb[:, 4 + j : 5 + j],
                    r_t[j][:, c0 : c0 + w], ALU.add, ALU.mult,
                )
                t2 = tmp.tile([P, w], BF16, name="t2", tag="t2")
                nc.vector.tensor_add(t2[:], t[:], ps_i[:, sl])
                nt = tmp.tile([P, w], BF16, name="nt", tag="nt")
                nc.scalar.activation(nt[:], t2[:], AFT.Tanh, bias=bn_sb[:, j : j + 1])
                d = tmp.tile([P, w], BF16, name="d", tag="d")
                nc.vector.tensor_sub(
                    d[:], U[:, bc, 4 + j, c0 - bc * BC : c0 - bc * BC + w], nt[:]
                )
                return nt, d

            def combine(j, c0, w, nt, d, ntsl, ring=None, veng=None):
                # h' = ng + z*(h - ng): only two serial DVE ops after z
                veng = veng or nc.vector
                zd = tmp.tile([P, w], BF16, name="zd", tag="zd")
                veng.tensor_mul(zd[:], z_t[j][:, c0 : c0 + w], d[:, ntsl])
                o_t = outp.tile([P, w], BF16, name="o_t", tag="o_t")
                veng.tensor_add(o_t[:], nt[:, ntsl], zd[:])
                (ring or nc.sync).dma_start(o_d[ts(j, P), c0 : c0 + w], o_t[:])

            def interleaved_group(j, bc):
                # first group only: k-major so each (U k-tile, 3 A rows)
                # chunk unlocks 3 matmuls in arrival order (PE queue is
                # in-order)
                ps_r = psum.tile([P, BC], F32, name="ps", tag="ps")
                ps_z = psum.tile([P, BC], F32, name="ps", tag="ps")
                ps_i = psum.tile([P, BC], F32, name="ps", tag="ps")
                ps_h = psum.tile([P, BC], F32, name="ps", tag="ps")

                def mm(ps, g, k, start, stop):
                    nc.tensor.matmul(ps[:], lhsT(g, k), U[:, bc, k, :],
                                     start=start, stop=stop)

                for k in range(8):
                    mm(ps_r, j, k, k == 0, k == 7)
                    mm(ps_z, 4 + j, k, k == 0, k == 7)
                    if k < 4:
                        mm(ps_i, 8 + j, k, k == 0, k == 3)
                    else:
                        mm(ps_h, 8 + j, k, k == 4, k == 7)
                return ps_r, ps_z, ps_i, ps_h

            # j-outer, bc-inner: group (j, bc1) reuses ALL of (j, bc0)'s A
            # rows, so the early groups' byte-per-flop demand drops below
            # the ~270GB/s the fill actually delivers (j0-bc0 needs 1.75MB,
            # j0-bc1 only +1MB of U, j1-bc1 nothing new, ...); bc-outer
            # needed A for j0..j3 (3MB) in the first 4 groups and stalled.
            for j in range(4):
                for bc in range(NBC):
                    last = bc == NBC - 1 and j == 3
                    if bc == 0 and j == 0:
                        ps_r, ps_z, ps_i, ps_h = interleaved_group(j, bc)
                        nc.scalar.activation(
                            r_t[j][:, ts(bc, BC)], ps_r[:], AFT.Sigmoid,
                            bias=brz_sb[:, j : j + 1],
                        )
                        nc.scalar.activation(
                            z_t[j][:, ts(bc, BC)], ps_z[:], AFT.Sigmoid,
                            bias=brz_sb[:, 4 + j : 5 + j],
                        )
                        nt, dd = make_nt(j, bc, bc * BC, BC, ps_i, ps_h, slice(0, BC))
                        combine(j, bc * BC, BC, nt, dd, slice(0, BC))
                        continue
                    ps_r = mm_group(j, bc, 0, BC, 0, KT)
                    nc.scalar.activation(
                        r_t[j][:, ts(bc, BC)], ps_r[:], AFT.Sigmoid,
                        bias=brz_sb[:, j : j + 1],
                    )
                    if not last:
                        ps_z = mm_group(4 + j, bc, 0, BC, 0, KT)
                        nc.scalar.activation(
                            z_t[j][:, ts(bc, BC)], ps_z[:], AFT.Sigmoid,
                            bias=brz_sb[:, 4 + j : 5 + j],
                        )
                        ps_h = mm_group(8 + j, bc, 0, BC, 4, KT)
                        ps_i = mm_group(8 + j, bc, 0, BC, 0, 4)
                        nt, dd = make_nt(j, bc, bc * BC, BC, ps_i, ps_h, slice(0, BC))
                        combine(j, bc * BC, BC, nt, dd, slice(0, BC))
                    else:
                        # final group: z gate LAST, in 128-wide quarters, so
                        # the chain trailing the very last matmul is a short
                        # sigmoid->mul->add->small-store. tanh/sub run in
                        # 256-halves so d's first half is ready before the
                        # first combine needs it; combines alternate
                        # Vector/GpSimd so the 8 elementwise ops don't
                        # serialize on one engine; stores alternate rings.
                        ps_h = mm_group(8 + j, bc, 0, BC, 4, KT)
                        ps_i = mm_group(8 + j, bc, 0, BC, 0, 4)
                        nts, dds = [], []
                        for hh in range(2):
                            hsl = slice(hh * 256, hh * 256 + 256)
                            nt_h, dd_h = make_nt(
                                j, bc, bc * BC + hh * 256, 256,
                                ps_i, ps_h, hsl,
                            )
                            nts.append(nt_h)
                            dds.append(dd_h)
                        # all quarter matmuls + sigmoids first (Scalar runs
                        # them back-to-back as MM groups complete), THEN the
                        # combines — a store on Scalar's FIFO ahead of a
                        # later sigmoid would block it
                        for q in range(4):
                            off = q * P
                            c0 = bc * BC + off
                            ps_z = mm_group(4 + j, bc, off, P, 0, KT)
                            nc.scalar.activation(
                                z_t[j][:, c0 : c0 + P], ps_z[:],
                                AFT.Sigmoid, bias=brz_sb[:, 4 + j : 5 + j],
                            )
                        for q in range(4):
                            off = q * P
                            c0 = bc * BC + off
                            sl = slice((q % 2) * P, (q % 2) * P + P)
                            ring = nc.scalar if q % 2 == 0 else nc.sync
                            combine(j, c0, P, nts[q // 2], dds[q // 2], sl,
                                    ring=ring)

    nc.compile()
    return nc


def _prep_core_inputs(x16, h16, W_ih16, W_hh16, b_ih, b_hh, n):
    a_full = np.concatenate([W_ih16[n].T, W_hh16[n].T], axis=0)      # (1024, 1536)
    a4 = a_full.reshape(KT, P, GT, P)                                # [k, kp, g, gp]
    a_rows = np.stack([a4[k, :, g, :] for (g, k) in ROW_ORDER])      # (96, P, P)
    a_re = np.ascontiguousarray(a_rows.transpose(1, 0, 2))           # (P, 96, P)
    u = np.ascontiguousarray(
        np.concatenate(
            [x16[:, n * BS : (n + 1) * BS].T, h16[:, n * BS : (n + 1) * BS].T],
            axis=0,
        ).reshape(KT, P, NBC, BC).transpose(1, 2, 0, 3)
    )                                                                # (P, NBC, KT, BC)
    brz8 = (b_ih[n, : 2 * BS] + b_hh[n, : 2 * BS]).reshape(8, P).T   # (P, 8)
    brz = np.ascontiguousarray(
        np.concatenate([brz8, -brz8[:, 4:8]], axis=1)
    )                                                                # (P, 12)
    bn = np.ascontiguousarray(
        np.concatenate(
            [b_ih[n, 2 * BS :].reshape(4, P).T, b_hh[n, 2 * BS :].reshape(4, P).T],
            axis=1,
        )
    )                                                                # (P, 8)
    return {"a": a_re, "u": u, "brz": brz, "bn": bn}


def kernel(x, h, W_ih, W_hh, b_ih, b_hh):
    global LAST_RESULTS
    import ml_dtypes

    bf16 = np.dtype(ml_dtypes.bfloat16)
    x16 = np.asarray(x, dtype=np.float32).astype(bf16)
    h16 = np.asarray(h, dtype=np.float32).astype(bf16)
    W_ih16 = np.asarray(W_ih, dtype=np.float32).astype(bf16)
    W_hh16 = np.asarray(W_hh, dtype=np.float32).astype(bf16)
    b_ih = np.asarray(b_ih, dtype=np.float32)
    b_hh = np.asarray(b_hh, dtype=np.float32)

    if "nc" not in _cache:
        _cache["nc"] = _build_nc()
    nc = _cache["nc"]

    in_maps = [
        _prep_core_inputs(x16, h16, W_ih16, W_hh16, b_ih, b_hh, n)
        for n in range(NB)
    ]
    trace = os.environ.get("BASS_KERNEL_TRACE") == "1"
    res = run_bass_kernel_spmd(nc, in_maps, list(range(NB)), trace=trace)
    LAST_RESULTS = res
    return np.concatenate(
        [res.results[n]["o"].astype(np.float32).T for n in range(NB)], axis=1
    )


# revision 20
# speedup vs baseline: 1.0604x; 1.0130x over previous
"""Block-diagonal GRU cell on 8 TRN2 NeuronCores — one block per core.

Math per block n (torch GRUCell):
  gi = x_n @ W_ih[n].T + b_ih[n]        (B, 3*BS)
  gh = h_n @ W_hh[n].T + b_hh[n]
  r = sigmoid(gi_r + gh_r); z = sigmoid(gi_z + gh_z)
  ng = tanh(gi_n + r * gh_n)
  h' = ng + z * (h_n - ng)

On-chip layout (per core): everything transposed on host so the
contraction (feature) dim is the SBUF partition dim and gates land on
PSUM partitions — biases then apply as per-partition ACT/DVE operands.
  A  = [W_ih[n].T ; W_hh[n].T]  -> (1024 feat, 1536 gates) bf16, cut
       into 96 (128x128) k-tiles laid out in DRAM in the EXACT order the
       PE consumes them, so every load is a contiguous prefix of the
       consumption stream and arbitrary chunk boundaries stay aligned
       with compute progress.
  U  = [x_n.T ; h_n.T]          -> (1024 feat, 1024 batch) bf16
  out = h'.T                    -> (512, 1024) bf16, un-transposed and
       upcast on host.
All matmuls run in bf16 (fp8 was measured 2.4-5x over the error budget
in simulation). r/z gates accumulate x- and h-matmuls into one PSUM
bank (8 k-steps); the n gate keeps i_n / h_n in separate banks.
Loads ride BOTH HWDGE rings: the Scalar (qActDynamicHW) ring carries
the first critical chunks (its engine clears the preamble barrier
~0.9us before Sync does) and the Sync (qSPDynamicHW) ring streams the
bulk — the two rings issue triggers in parallel so small early chunks
don't serialize behind one engine's ~0.65us/trigger cost. Early chunks
are small (128-384KB) to make the first matmul's data land ASAP; later
chunks are big for bandwidth. Warm-up matmuls (N=128 off a memset
tile) keep the PE HAM activity window busy from the top of the kernel
until real data lands, so the 1.2->2.4GHz un-throttle fires during the
fill instead of 8us into the real stream. The final row-block group
runs its z-gate matmuls last in 128-wide quarters so the chain
trailing the very last matmul is sigmoid->mul->add->small-store, with
stores alternating rings.
"""

import os
import sys

import numpy as np

try:
    import concourse.bass as bass
except ImportError:  # fresh grading dir: fall back to the repo checkout
    sys.path.insert(0, "/opt/trn_rl_repo")
    import concourse.bass as bass

import concourse.mybir as mybir
import concourse.tile as tile
from concourse import bacc
from concourse.bass import ts
from concourse.bass_utils import run_bass_kernel_spmd

B = 1024            # batch
NB = 8              # blocks == cores
BS = 512            # hidden block size
G3 = 3 * BS         # gates per block (r, z, n)
KF = 1024           # contraction feats per core: 512 input + 512 hidden
P = 128
KT = KF // P        # 8 k-tiles
GT = G3 // P        # 12 gate column groups: 0-3 r, 4-7 z, 8-11 n
NBC = 2             # batch chunks
BC = B // NBC       # 512 (one PSUM bank of fp32)
NROW = GT * KT      # 96 (128x128) A tiles

F32 = mybir.dt.float32
BF16 = mybir.dt.bfloat16
AFT = mybir.ActivationFunctionType
ALU = mybir.AluOpType

_cache: dict = {}
LAST_RESULTS = None  # BassKernelResults of the most recent run (for test.py)


def _row_order():
    """(gate_group, k) per DRAM row of A, in exact PE consumption order.

    j=0 runs k-major (r,z,n per k-tile) so each 128KB U k-tile plus
    three 32KB A rows unlocks three matmuls — the finest-grained
    data->compute pipeline the DMA sem granularity allows; j=1..3 run
    r(k0-7), z(k0-7), n(k4-7), n(k0-3) — n's k4-7 half first because
    the tanh chain consumes the h-side accumulation before the i-side.
    """
    rows = []
    for k in range(8):
        rows += [(0, k), (4, k), (8, k)]
    for j in (1, 2, 3):
        rows += [(j, k) for k in range(8)]
        rows += [(4 + j, k) for k in range(8)]
        rows += [(8 + j, k) for k in range(4, 8)]
        rows += [(8 + j, k) for k in range(4)]
    assert len(rows) == NROW
    return rows


ROW_ORDER = _row_order()
ROW_IDX = {gk: i for i, gk in enumerate(ROW_ORDER)}

# Fill rides TWO descriptor rings whose streams are CO-critical and
# aligned with the interleaved consumption: the U k-tiles on the Sync
# HWDGE ring, the j0 A rows on the GpSimd SWDGE ring (its engine clears
# the preamble barrier ~0.6us before Sync). Per-chunk trigger (~0.65us)
# and completion-receipt (~0.7us) overheads then pipeline in parallel,
# which single-ring delivery (~250-300GB/s effective vs the 346GB/s the
# warm j0 stream consumes) could not sustain. Bulk-starving-the-front
# (the earlier 2-ring failure) doesn't apply because both rings carry
# front-of-stream data in matching order. j1..j3 A chunks go on Sync
# after the U k-tiles; chunk sizes grow once the stream is ahead.
# A j0 rows ride the Scalar HWDGE ring (qActDynamicHW, ~0.6us fixed
# cost, idle until the first sigmoid ~13us); SWDGE (GpSimd) measured
# ~2us fixed per DMA and starved the j0 A rows. U k-tiles ride Sync.
# Both rings carry front-of-stream data in matched consumption order.
SCALAR_LOADS = [
    ("a", 0, 3), ("a", 3, 6), ("a", 6, 12), ("a", 12, 18), ("a", 18, 24),
]
SYNC_LOADS = [
    ("u", 0, 0, 1), ("u", 0, 1, 2), ("u", 0, 2, 4), ("u", 0, 4, 6),
    ("u", 0, 6, 8),
    ("u", 1, 0, 2), ("u", 1, 2, 4), ("u", 1, 4, 6), ("u", 1, 6, 8),
    ("a", 24, 36), ("a", 36, 48), ("a", 48, 72), ("a", 72, 96),
]

# Bridge PE activity from barrier-exit (~7.4us abs) past first-data
# (~9.5us abs) with margin — an idle hole here resets the HAM activity
# window and costs ~4us of 1.2GHz matmuls (measured).
N_WARMUP = 26


def _build_nc():
    nc = bacc.Bacc("TRN2", target_bir_lowering=False, debug=False, num_devices=NB)
    a_d = nc.dram_tensor("a", [P, NROW, P], BF16, kind="ExternalInput").ap()
    u_d = nc.dram_tensor("u", [P, NBC, KT, BC], BF16, kind="ExternalInput").ap()
    brz_d = nc.dram_tensor("brz", [P, 12], F32, kind="ExternalInput").ap()
    bn_d = nc.dram_tensor("bn", [P, 8], F32, kind="ExternalInput").ap()
    o_d = nc.dram_tensor("o", [BS, B], BF16, kind="ExternalOutput").ap()

    with tile.TileContext(nc) as tc:
        with (
            tc.tile_pool(name="persist", bufs=1) as persist,
            tc.tile_pool(name="tmp", bufs=4) as tmp,
            tc.tile_pool(name="outp", bufs=4) as outp,
            tc.tile_pool(name="psum", bufs=8, space="PSUM") as psum,
        ):
            # Warm-up source: memset on Vector (fast, idle; a GpSimd memset
            # of [128,128] measured ~1.2us and serialized both the warm-ups
            # and the GpSimd A-ring triggers behind it).
            wsb = persist.tile([P, P], BF16, name="wsb")
            nc.vector.memset(wsb[:], 0.0)

            U = persist.tile([P, NBC, KT, BC], BF16, name="U")
            A = persist.tile([P, NROW, P], BF16, name="A")

            def emit_loads(eng, chunks):
                for c in chunks:
                    if c[0] == "a":
                        _, r0, r1 = c
                        eng.dma_start(A[:, r0:r1, :], a_d[:, r0:r1, :])
                    else:
                        _, bcq, k0, k1 = c
                        eng.dma_start(U[:, bcq, k0:k1, :], u_d[:, bcq, k0:k1, :])

            emit_loads(nc.scalar, SCALAR_LOADS)
            emit_loads(nc.sync, SYNC_LOADS)

            # tiny bias loads ride the GpSimd SWDGE queue after the
            # critical j0 A rows; first needed at j0's sigmoid (~13us)
            brz_sb = persist.tile([P, 12], F32, name="brz_sb")
            nc.gpsimd.dma_start(brz_sb[:], brz_d[:])
            bn_sb = persist.tile([P, 8], F32, name="bn_sb")
            nc.gpsimd.dma_start(bn_sb[:], bn_d[:])

            # PE warm-up matmuls: cover the HAM activity window + data-fill
            # lead so the first real matmuls run at 2.4GHz.
            wps = psum.tile([P, BC], F32, name="wps", tag="ps")
            for _ in range(N_WARMUP):
                nc.tensor.matmul(wps[:, :P], wsb[:], wsb[:], start=True, stop=True)

            def lhsT(g, k):
                return A[:, ROW_IDX[(g, k)], :]

            # persistent per row-block j: r gate, z gate (bf16)
            r_t = [persist.tile([P, B], BF16, name=f"r{j}") for j in range(4)]
            z_t = [persist.tile([P, B], BF16, name=f"z{j}") for j in range(4)]

            def mm_group(g, bc, c0, w, k0, k1):
                ps = psum.tile([P, w], F32, name="ps", tag="ps")
                for k in range(k0, k1):
                    nc.tensor.matmul(
                        ps[:],
                        lhsT(g, k),
                        U[:, bc, k, c0 : c0 + w],
                        start=(k == k0),
                        stop=(k == k1 - 1),
                    )
                return ps

            def make_nt(j, bc, c0, w, ps_i, ps_h, sl):
                # ng = tanh(i_n + b_in + r*(h_n + b_hn)); d = h - ng
                t = tmp.tile([P, w], F32, name="t", tag="t")
                nc.vector.scalar_tensor_tensor(
                    t[:], ps_h[:, sl], bn_sb[:, 4 + j : 5 + j],
                    r_t[j][:, c0 : c0 + w], ALU.add, ALU.mult,
                )
                t2 = tmp.tile([P, w], BF16, name="t2", tag="t2")
                nc.vector.tensor_add(t2[:], t[:], ps_i[:, sl])
                nt = tmp.tile([P, w], BF16, name="nt", tag="nt")
                nc.scalar.activation(nt[:], t2[:], AFT.Tanh, bias=bn_sb[:, j : j + 1])
                d = tmp.tile([P, w], BF16, name="d", tag="d")
                nc.vector.tensor_sub(
                    d[:], U[:, bc, 4 + j, c0 - bc * BC : c0 - bc * BC + w], nt[:]
                )
                return nt, d

            def combine(j, c0, w, nt, d, ntsl, ring=None, veng=None):
                # h' = ng + z*(h - ng): only two serial DVE ops after z
                veng = veng or nc.vector
                zd = tmp.tile([P, w], BF16, name="zd", tag="zd")
                veng.tensor_mul(zd[:], z_t[j][:, c0 : c0 + w], d[:, ntsl])
                o_t = outp.tile([P, w], BF16, name="o_t", tag="o_t")
                veng.tensor_add(o_t[:], nt[:, ntsl], zd[:])
                (ring or nc.sync).dma_start(o_d[ts(j, P), c0 : c0 + w], o_t[:])

            def interleaved_group(j, bc):
                # first group only: k-major so each (U k-tile, 3 A rows)
                # chunk unlocks 3 matmuls in arrival order (PE queue is
                # in-order)
                ps_r = psum.tile([P, BC], F32, name="ps", tag="ps")
                ps_z = psum.tile([P, BC], F32, name="ps", tag="ps")
                ps_i = psum.tile([P, BC], F32, name="ps", tag="ps")
                ps_h = psum.tile([P, BC], F32, name="ps", tag="ps")

                def mm(ps, g, k, start, stop):
                    nc.tensor.matmul(ps[:], lhsT(g, k), U[:, bc, k, :],
                                     start=start, stop=stop)

                for k in range(8):
                    mm(ps_r, j, k, k == 0, k == 7)
                    mm(ps_z, 4 + j, k, k == 0, k == 7)
                    if k < 4:
                        mm(ps_i, 8 + j, k, k == 0, k == 3)
                    else:
                        mm(ps_h, 8 + j, k, k == 4, k == 7)
                return ps_r, ps_z, ps_i, ps_h

            # j-outer, bc-inner: group (j, bc1) reuses ALL of (j, bc0)'s A
            # rows, so the early groups' byte-per-flop demand drops below
            # the ~270GB/s the fill actually delivers (j0-bc0 needs 1.75MB,
            # j0-bc1 only +1MB of U, j1-bc1 nothing new, ...); bc-outer
            # needed A for j0..j3 (3MB) in the first 4 groups and stalled.
            for j in range(4):
                for bc in range(NBC):
                    last = bc == NBC - 1 and j == 3
                    if bc == 0 and j == 0:
                        ps_r, ps_z, ps_i, ps_h = interleaved_group(j, bc)
                        nc.scalar.activation(
                            r_t[j][:, ts(bc, BC)], ps_r[:], AFT.Sigmoid,
                            bias=brz_sb[:, j : j + 1],
                        )
                        nc.scalar.activation(
                            z_t[j][:, ts(bc, BC)], ps_z[:], AFT.Sigmoid,
                            bias=brz_sb[:, 4 + j : 5 + j],
                        )
                        nt, dd = make_nt(j, bc, bc * BC, BC, ps_i, ps_h, slice(0, BC))
                        combine(j, bc * BC, BC, nt, dd, slice(0, BC))
                        continue
                    ps_r = mm_group(j, bc, 0, BC, 0, KT)
                    nc.scalar.activation(
                        r_t[j][:, ts(bc, BC)], ps_r[:], AFT.Sigmoid,
                        bias=brz_sb[:, j : j + 1],
                    )
                    if not last:
                        ps_z = mm_group(4 + j, bc, 0, BC, 0, KT)
                        nc.scalar.activation(
                            z_t[j][:, ts(bc, BC)], ps_z[:], AFT.Sigmoid,
                            bias=brz_sb[:, 4 + j : 5 + j],
                        )
                        ps_h = mm_group(8 + j, bc, 0, BC, 4, KT)
                        ps_i = mm_group(8 + j, bc, 0, BC, 0, 4)
                        nt, dd = make_nt(j, bc, bc * BC, BC, ps_i, ps_h, slice(0, BC))
                        combine(j, bc * BC, BC, nt, dd, slice(0, BC))
                    else:
                        # final group: z gate LAST, in 128-wide quarters, so
                        # the chain trailing the very last matmul is a short
                        # sigmoid->mul->add->small-store. tanh/sub run in
                        # 256-halves so d's first half is ready before the
                        # first combine needs it; combines alternate
                        # Vector/GpSimd so the 8 elementwise ops don't
                        # serialize on one engine; stores alternate rings.
                        ps_h = mm_group(8 + j, bc, 0, BC, 4, KT)
                        ps_i = mm_group(8 + j, bc, 0, BC, 0, 4)
                        nts, dds = [], []
                        for hh in range(2):
                            hsl = slice(hh * 256, hh * 256 + 256)
                            nt_h, dd_h = make_nt(
                                j, bc, bc * BC + hh * 256, 256,
                                ps_i, ps_h, hsl,
                            )
                            nts.append(nt_h)
                            dds.append(dd_h)
                        # all quarter matmuls + sigmoids first (Scalar runs
                        # them back-to-back as MM groups complete), THEN the
                        # combines — a store on Scalar's FIFO ahead of a
                        # later sigmoid would block it
                        for q in range(4):
                            off = q * P
                            c0 = bc * BC + off
                            ps_z = mm_group(4 + j, bc, off, P, 0, KT)
                            nc.scalar.activation(
                                z_t[j][:, c0 : c0 + P], ps_z[:],
                                AFT.Sigmoid, bias=brz_sb[:, 4 + j : 5 + j],
                            )
                        for q in range(4):
                            off = q * P
                            c0 = bc * BC + off
                            sl = slice((q % 2) * P, (q % 2) * P + P)
                            ring = nc.scalar if q % 2 == 0 else nc.sync
                            combine(j, c0, P, nts[q // 2], dds[q // 2], sl,
                                    ring=ring)

    nc.compile()
    return nc


def _prep_core_inputs(x16, h16, W_ih16, W_hh16, b_ih, b_hh, n):
    a_full = np.concatenate([W_ih16[n].T, W_hh16[n].T], axis=0)      # (1024, 1536)
    a4 = a_full.reshape(KT, P, GT, P)                                # [k, kp, g, gp]
    a_rows = np.stack([a4[k, :, g, :] for (g, k) in ROW_ORDER])      # (96, P, P)
    a_re = np.ascontiguousarray(a_rows.transpose(1, 0, 2))           # (P, 96, P)
    u = np.ascontiguousarray(
        np.concatenate(
            [x16[:, n * BS : (n + 1) * BS].T, h16[:, n * BS : (n + 1) * BS].T],
            axis=0,
        ).reshape(KT, P, NBC, BC).transpose(1, 2, 0, 3)
    )                                                                # (P, NBC, KT, BC)
    brz8 = (b_ih[n, : 2 * BS] + b_hh[n, : 2 * BS]).reshape(8, P).T   # (P, 8)
    brz = np.ascontiguousarray(
        np.concatenate([brz8, -brz8[:, 4:8]], axis=1)
    )                                                                # (P, 12)
    bn = np.ascontiguousarray(
        np.concatenate(
            [b_ih[n, 2 * BS :].reshape(4, P).T, b_hh[n, 2 * BS :].reshape(4, P).T],
            axis=1,
        )
    )                                                                # (P, 8)
    return {"a": a_re, "u": u, "brz": brz, "bn": bn}


def kernel(x, h, W_ih, W_hh, b_ih, b_hh):
    global LAST_RESULTS
    import ml_dtypes

    bf16 = np.dtype(ml_dtypes.bfloat16)
    x16 = np.asarray(x, dtype=np.float32).astype(bf16)
    h16 = np.asarray(h, dtype=np.float32).astype(bf16)
    W_ih16 = np.asarray(W_ih, dtype=np.float32).astype(bf16)
    W_hh16 = np.asarray(W_hh, dtype=np.float32).astype(bf16)
    b_ih = np.asarray(b_ih, dtype=np.float32)
    b_hh = np.asarray(b_hh, dtype=np.float32)

    if "nc" not in _cache:
        _cache["nc"] = _build_nc()
    nc = _cache["nc"]

    in_maps = [
        _prep_core_inputs(x16, h16, W_ih16, W_hh16, b_ih, b_hh, n)
        for n in range(NB)
    ]
    trace = os.environ.get("BASS_KERNEL_TRACE") == "1"
    res = run_bass_kernel_spmd(nc, in_maps, list(range(NB)), trace=trace)
    LAST_RESULTS = res
    return np.concatenate(
        [res.results[n]["o"].astype(np.float32).T for n in range(NB)], axis=1
    )
